# revision 1
# baseline (speedup 1.0000x reference)
"""KGramEmbeddingMLP on 8 TRN2 NeuronCores.

Model: one-hot context [256, 8*50257] -> embedding lookup (dense one-hot
matmul) -> MLP 512->1024->1024 (silu) -> vocab head 1024->50257.

Sharding:
  Phase 1+2 data-parallel over batch (32 rows/core): each core streams its
  transposed one-hot slab through the TensorEngine against the embed table,
  then runs the small MLP.
  AllGather of h2 (64KB/core), then phase 3 tensor-parallel over vocab:
  each core computes logits[:, c*VS:(c+1)*VS] from an SBUF-resident W3 shard.

dtypes: context/embed/W1/W2/W3/h1/h2 in bf16 (one-hot 0/1 and the embedded
values are exact in bf16), all PSUM accumulation f32, logits f32.

Layout: the context is host-transposed and pre-blocked so every streaming
DMA is one fully contiguous 512KB block ([128 partitions x 4KB]).  ctx
DMAs ride the sync HWDGE ring, everything else the scalar ring.
"""

import numpy as np
import ml_dtypes

VOCAB = 50257
K = 8
EMBED = 64
HIDDEN = 1024
BATCH = 256
NCORES = 8

VP = 51200              # vocab padded to 400*128 (uniform 8-tile DMA blocks)
VT = VP // 128          # 400 contraction tiles
CB = 8                  # ctx v-tiles per DMA block
NQ = VT // CB           # 50 ctx blocks
EBLK = 40               # v-tiles per emb DMA block (10 blocks, CB-aligned)
BPC = BATCH // NCORES   # 32 batch rows per core
ROWS = BPC * K          # 256 (b,k) rows per core; column index = b*8 + k
VS = VP // NCORES       # 6400 head columns per core

BF16 = ml_dtypes.bfloat16

TRACE = False           # test.py sets this to capture a neuron profile
LAST_RESULT = None      # BassKernelResults from the most recent run

_compiled = {}


def _head_chunks():
    chunks = []
    off = 0
    while off < VS:
        w = min(512, VS - off)
        chunks.append((off, w))
        off += w
    return chunks


def _build():
    import concourse.mybir as mybir
    import concourse.tile as tile
    from concourse import bacc

    f32 = mybir.dt.float32
    bf16 = mybir.dt.bfloat16

    nc = bacc.Bacc(
        "TRN2", target_bir_lowering=False, debug=False, num_devices=NCORES
    )

    ctx_d = nc.dram_tensor("ctxT", [NQ, 128, CB * ROWS], bf16, kind="ExternalInput")
    emb_d = nc.dram_tensor("emb", [VT // EBLK, 128, EBLK * EMBED], bf16, kind="ExternalInput")
    w1_d = nc.dram_tensor("w1", [K * EMBED, HIDDEN], bf16, kind="ExternalInput")
    b1_d = nc.dram_tensor("b1t", [128, HIDDEN // 128], f32, kind="ExternalInput")
    w2_d = nc.dram_tensor("w2", [HIDDEN, HIDDEN], bf16, kind="ExternalInput")
    b2_d = nc.dram_tensor("b2t", [128, HIDDEN // 128], f32, kind="ExternalInput")
    w3_d = nc.dram_tensor("w3", [HIDDEN, VS], bf16, kind="ExternalInput")
    b3_d = nc.dram_tensor("b3", [1, VS], bf16, kind="ExternalInput")
    out_d = nc.dram_tensor("out", [BATCH, VS], f32, kind="ExternalOutput")

    KT1 = (K * EMBED) // 128   # 4 contraction tiles for W1
    KT2 = HIDDEN // 128        # 8 contraction tiles for W2 / W3
    MT = HIDDEN // 128         # 8 output tiles for h1/h2

    with tile.TileContext(nc) as tc:
        with (
            tc.tile_pool(name="const", bufs=1) as const,
            tc.tile_pool(name="stream", bufs=6) as stream,
            tc.tile_pool(name="embp", bufs=3) as embp,
            tc.tile_pool(name="mlp", bufs=2) as mlp,
            tc.tile_pool(name="head", bufs=3) as head,
            tc.tile_pool(name="psum1", bufs=1, space="PSUM") as psum1,
            tc.tile_pool(name="psum", bufs=2, space="PSUM") as psum,
            tc.tile_pool(name="psum_o", bufs=4, space="PSUM") as psum_o,
            tc.tile_pool(name="dram", bufs=1, space="DRAM") as dram,
        ):
            # ---- resident weights (scalar HWDGE ring) -----------------
            w1_sb = []
            for kk in range(KT1):
                t = const.tile([128, HIDDEN], bf16, tag=f"w1_{kk}")
                nc.gpsimd.dma_start(t[:], w1_d[kk * 128:(kk + 1) * 128, :])
                w1_sb.append(t)
            w2_sb = []
            for kk in range(KT2):
                t = const.tile([128, HIDDEN], bf16, tag=f"w2_{kk}")
                nc.gpsimd.dma_start(t[:], w2_d[kk * 128:(kk + 1) * 128, :])
                w2_sb.append(t)
            w3_sb = []
            for kk in range(KT2):
                t = const.tile([128, VS], bf16, tag=f"w3_{kk}")
                if kk < 0:
                    nc.gpsimd.dma_start(t[:], w3_d[kk * 128:(kk + 1) * 128, :])
                w3_sb.append(t)
            b1_sb = const.tile([128, HIDDEN // 128], f32, tag="b1")
            nc.gpsimd.dma_start(b1_sb[:], b1_d[:])
            b2_sb = const.tile([128, HIDDEN // 128], f32, tag="b2")
            nc.gpsimd.dma_start(b2_sb[:], b2_d[:])
            b3_sb = const.tile([1, VS], bf16, tag="b3")
            nc.gpsimd.dma_start(b3_sb[:], b3_d[:])
            b3b_sb = const.tile([128, VS], bf16, tag="b3b")
            nc.gpsimd.partition_broadcast(b3b_sb[:], b3_sb[:])

            # ---- phase 1: embedded^T[64, 256] = emb^T @ ctxT ----------
            emb_t = psum1.tile([EMBED, ROWS], f32, tag="embT")
            for q in range(NQ):
                ctile = stream.tile([128, CB * ROWS], bf16, tag="ctx")
                ctx_eng = nc.sync if (q % 5) < 3 else nc.scalar
                ctx_eng.dma_start(ctile[:], ctx_d[q])
                if q % (EBLK // CB) == 0:
                    eq = q // (EBLK // CB)
                    etile = embp.tile([128, EBLK * EMBED], bf16, tag="emb")
                    nc.scalar.dma_start(etile[:], emb_d[eq])
                for i in range(CB):
                    jj = q * CB + i
                    n = jj % EBLK
                    nc.tensor.matmul(
                        emb_t[:],
                        etile[:, n * EMBED:(n + 1) * EMBED],
                        ctile[:, i * ROWS:(i + 1) * ROWS],
                        start=(jj == 0),
                        stop=(jj == VT - 1),
                    )

            # ---- rearrange embedded -> xT [512, 32] (4 tiles, bf16) ---
            # emb_t free index = b*8 + k ; xT partition = k*64 + e
            embs = mlp.tile([EMBED, ROWS], bf16, tag="embs")
            nc.vector.tensor_copy(embs[:], emb_t[:])
            embs_r = embs[:].rearrange("e (b k) -> e k b", k=K)
            xt = []
            for t_i in range(KT1):
                t = mlp.tile([128, BPC], bf16, tag=f"xt_{t_i}")
                xt.append(t)
            for k in range(K):
                dst = xt[k // 2]
                p0 = 64 * (k % 2)
                nc.sync.dma_start(dst[p0:p0 + 64, :], embs_r[:, k, :])

            # ---- phase 2: h1 = silu(x@W1+b1); h2 = silu(h1@W2+b2) -----
            h1t = []
            for m in range(MT):
                ps = psum.tile([128, BPC], f32, tag="ps_mlp")
                for kk in range(KT1):
                    nc.tensor.matmul(
                        ps[:],
                        w1_sb[kk][:, m * 128:(m + 1) * 128],
                        xt[kk][:],
                        start=(kk == 0),
                        stop=(kk == KT1 - 1),
                    )
                t = mlp.tile([128, BPC], bf16, tag=f"h1_{m}")
                nc.scalar.activation(
                    t[:], ps[:],
                    mybir.ActivationFunctionType.Silu,
                    bias=b1_sb[:, m:m + 1],
                )
                h1t.append(t)

            cc_in = dram.tile([HIDDEN, BPC], bf16, tag="cc_in")
            for m in range(MT):
                ps = psum.tile([128, BPC], f32, tag="ps_mlp")
                for kk in range(KT2):
                    nc.tensor.matmul(
                        ps[:],
                        w2_sb[kk][:, m * 128:(m + 1) * 128],
                        h1t[kk][:],
                        start=(kk == 0),
                        stop=(kk == KT2 - 1),
                    )
                t = mlp.tile([128, BPC], bf16, tag=f"h2_{m}")
                nc.scalar.activation(
                    t[:], ps[:],
                    mybir.ActivationFunctionType.Silu,
                    bias=b2_sb[:, m:m + 1],
                )
                nc.sync.dma_start(cc_in[m * 128:(m + 1) * 128, :], t[:])

            # ---- all-gather h2 across the 8 cores ---------------------
            cc_out = dram.tile(
                [NCORES * HIDDEN, BPC], bf16, tag="cc_out", addr_space="Shared"
            )
            cc = nc.gpsimd.collective_compute(
                "AllGather",
                mybir.AluOpType.bypass,
                replica_groups=[list(range(NCORES))],
                ins=[cc_in[:].opt()],
                outs=[cc_out[:].opt()],
            )
            from concourse.bass import _add_dep_helper
            for kk in range(KT2):
                w3dma = nc.gpsimd.dma_start(
                    w3_sb[kk][:], w3_d[kk * 128:(kk + 1) * 128, :]
                )
                _add_dep_helper(
                    w3dma.ins, cc.ins, False, "fill AG dead window with W3"
                )

            # ---- load h2_full^T [1024, 256] (8 tiles, bf16) -----------
            cc_r = cc_out[:].rearrange("(c kk p) b -> kk p c b", kk=KT2, p=128)
            h2f = []
            for kk in range(KT2):
                t = mlp.tile([128, BATCH], bf16, tag=f"h2f_{kk}")
                nc.sync.dma_start(
                    t[:].rearrange("p (c b) -> p c b", b=BPC), cc_r[kk]
                )
                h2f.append(t)

            # ---- phase 3: logits[:, shard] = h2_full @ W3s + b3s ------
            for off, w in _head_chunks():
                for r in range(BATCH // 128):
                    ps = psum_o.tile([128, 512], f32, tag="ps_out")
                    for kk in range(KT2):
                        nc.tensor.matmul(
                            ps[:, :w],
                            h2f[kk][:, r * 128:(r + 1) * 128],
                            w3_sb[kk][:, off:off + w],
                            start=(kk == 0),
                            stop=(kk == KT2 - 1),
                        )
                    osb = head.tile([128, 512], f32, tag="osb")
                    nc.vector.tensor_add(osb[:, :w], ps[:, :w], b3b_sb[:, off:off + w])
                    nc.sync.dma_start(
                        out_d[r * 128:(r + 1) * 128, off:off + w], osb[:, :w]
                    )

    nc.compile()
    return nc


def _get_nc():
    if "nc" not in _compiled:
        _compiled["nc"] = _build()
    return _compiled["nc"]


def _prep_inputs(context_flat, embed_w, W1, b1, W2, b2, W3, b3):
    ctx3 = np.asarray(context_flat, np.float32).reshape(BATCH, K, VOCAB)

    emb_p = np.zeros((VP, EMBED), BF16)
    emb_p[:VOCAB] = np.asarray(embed_w, np.float32).astype(BF16)
    # emb blocks: [8, 128, EBLK*EMBED], block eq = v-tiles [eq*EBLK, (eq+1)*EBLK)
    nebq = VT // EBLK
    emb_b = np.ascontiguousarray(
        emb_p.reshape(nebq, EBLK, 128, EMBED).swapaxes(1, 2)
    ).reshape(nebq, 128, EBLK * EMBED)

    w1 = np.asarray(W1, np.float32).astype(BF16)
    w2 = np.asarray(W2, np.float32).astype(BF16)
    b1t = np.ascontiguousarray(np.asarray(b1, np.float32).reshape(MT_R, 128).T)
    b2t = np.ascontiguousarray(np.asarray(b2, np.float32).reshape(MT_R, 128).T)

    w3_p = np.zeros((HIDDEN, VP), BF16)
    w3_p[:, :VOCAB] = np.asarray(W3, np.float32).astype(BF16)
    b3_p = np.zeros((1, VP), BF16)
    b3_p[0, :VOCAB] = np.asarray(b3, np.float32).astype(BF16)

    in_maps = []
    for c in range(NCORES):
        src = ctx3[c * BPC:(c + 1) * BPC].reshape(ROWS, VOCAB)
        ctxT = np.zeros((VP, ROWS), BF16)
        ctxT[:VOCAB] = src.astype(BF16).T
        ctx_b = np.ascontiguousarray(
            ctxT.reshape(NQ, CB, 128, ROWS).swapaxes(1, 2)
        ).reshape(NQ, 128, CB * ROWS)
        in_maps.append({
            "ctxT": ctx_b,
            "emb": emb_b,
            "w1": w1,
            "b1t": b1t,
            "w2": w2,
            "b2t": b2t,
            "w3": np.ascontiguousarray(w3_p[:, c * VS:(c + 1) * VS]),
            "b3": np.ascontiguousarray(b3_p[:, c * VS:(c + 1) * VS]),
        })
    return in_maps


MT_R = HIDDEN // 128


def kernel(**inputs):
    global LAST_RESULT
    from concourse import bass_utils

    nc = _get_nc()
    in_maps = _prep_inputs(**inputs)
    res = bass_utils.run_bass_kernel_spmd(
        nc, in_maps, core_ids=list(range(NCORES)), trace=TRACE
    )
    LAST_RESULT = res
    full = np.empty((BATCH, VP), np.float32)
    for c in range(NCORES):
        full[:, c * VS:(c + 1) * VS] = res.results[c]["out"]
    return np.ascontiguousarray(full[:, :VOCAB])

